# revision 55
# baseline (speedup 1.0000x reference)
"""Trainium2 Bass kernel for the dual-branch spatial-reduction attention module.

Sharding (8 NeuronCores): branch-parallel (cores 0-3 -> branch 0, cores 4-7 ->
branch 1); within a branch quad, query-token-parallel (each core owns 1024 of
the 4096 query tokens). The spatial-reduction conv + LayerNorm + k/v
projections over the 1024 kv positions are replicated on every core (cheaper
than cross-core collectives on this platform). The host does layout prep
(transposes, dtype casts, weight folding/permutation, conv patch gathering)
and the final TokenExchange between branches.

Per-core program:
  - conv = 4 shifted matmuls over host-pregathered 2x2 patches (bf16,
    token-major, 8 chunks of 128 positions) + bias via a K=1 ones-matmul;
    LayerNorm fused on the PSUM output; rstd computed as exp(-0.5*ln(var+eps))
    so every Activation-engine op stays in the natural_log_exp table (the
    softmax exp shares it -> zero table reloads); xn transposed to
    channel-major via bf16 DMA-XBAR tiles
  - q projection in fp8 DoubleRow (K=256 per instruction, 0.5 cycles/row);
    the softmax scale is folded into Wq and Wq's columns are host-permuted so
    the PSUM partitions land directly in the (head, hd-half) layout the fp8
    qk wants; PSUM staged to fp8 qT8 on the DVE
  - k projection bf16 per-mc-chunk (gates the qk stream one conv chunk behind
    the conv), PSUM staged to fp8 kT8; the k bias is dropped (softmax cancels
    it) and the v bias is folded into the output-projection bias
  - qk in fp8 DoubleRow: per (head-pair, mc) one [128,1024] PSUM tile from
    two K=32x2 matmuls; softmax exp on the Activation engine streams from
    conv chunk 1 onward so the exp engine (the non-PE bottleneck) is
    saturated almost the whole kernel
  - attn@v re-oriented: lhsT = pt chunk [128m, 128n], rhs = v [128m, 65]
    (ones column appended for the softmax denominator) -> out [128n, 65]
    PSUM: 65 free cycles per matmul instead of 512, halving attn@v PE time;
    per-(head, n-chunk) normalization is two per-partition DVE ops
    (reciprocal of the denominator column + scale), with the output mask
    folded into the scale for free
  - attn tiles DMA-transposed to channel-major; out projection per n-chunk
    with bias applied as a mask-row (x) bias-row K=1 outer-product matmul
    (mask already folded into attn); PSUM drained to SBUF on the Pool engine
    and stored via the SP DMA queue
"""

import sys

sys.path.insert(0, "/opt/trn_rl_repo")

import numpy as np
import ml_dtypes

BF16 = ml_dtypes.bfloat16
# dt.float8e4 decodes as IEEE-style e4m3 (bias 8, max 240) per concourse/dt.py
FP8 = ml_dtypes.float8_e4m3
# q is ~8x smaller than k; shifting sqrt(8) of scale from k to q puts both
# at std ~0.16, clear of the e4m3 subnormal floor (the product is invariant)
QK_BAL = np.sqrt(8.0)

NUM_HEADS = 8
SR = 2
LN_EPS = 1e-5
MASK_THRESHOLD = 0.02
B, N, C = 1, 4096, 512
H = W = 64
M = N // (SR * SR)  # 1024 kv positions
HD = C // NUM_HEADS  # 64
NQ = N // 4  # 1024 query tokens per core
N_CORES = 8

_compiled = None


def _build():
    import concourse.bass as bass
    import concourse.tile as tile
    from concourse import bacc, mybir

    f32 = mybir.dt.float32
    bf16 = mybir.dt.bfloat16
    fp8 = mybir.dt.float8e4
    DR = mybir.MatmulPerfMode.DoubleRow

    nc = bacc.Bacc("TRN2", target_bir_lowering=False, debug=False,
                   num_devices=N_CORES)

    # ---- DRAM I/O ----
    # conv inputs/weights as fp8 hi/lo splits (wsr prescaled x16 on the host;
    # LayerNorm absorbs the uniform scale): [p, ij, cch, t, .]
    xph_d = nc.dram_tensor("xp_hi", [128, 4, 2, 2, M], fp8,
                           kind="ExternalInput")
    xpl_d = nc.dram_tensor("xp_lo", [128, 4, 2, 2, M], fp8,
                           kind="ExternalInput")
    wsh_d = nc.dram_tensor("wsr_hi", [128, 4, 2, 2, C], fp8,
                           kind="ExternalInput")
    wsl_d = nc.dram_tensor("wsr_lo", [128, 4, 2, 2, C], fp8,
                           kind="ExternalInput")
    xq_d = nc.dram_tensor("xqT", [C, NQ], bf16, kind="ExternalInput")
    wq_d = nc.dram_tensor("wq", [C, C], bf16, kind="ExternalInput")
    bsr_d = nc.dram_tensor("bsr_r", [1, C], bf16, kind="ExternalInput")
    wk_d = nc.dram_tensor("wk", [C, C], bf16, kind="ExternalInput")
    wv_d = nc.dram_tensor("wv", [C, C], bf16, kind="ExternalInput")
    wp_d = nc.dram_tensor("wp", [C, C], bf16, kind="ExternalInput")
    bp_d = nc.dram_tensor("bp_r", [1, C], bf16, kind="ExternalInput")
    mask_d = nc.dram_tensor("mask_s", [128, 8], f32, kind="ExternalInput")
    maskr_d = nc.dram_tensor("mask_r", [1, 8, 128], bf16, kind="ExternalInput")
    out_d = nc.dram_tensor("out", [NQ, C], bf16, kind="ExternalOutput")

    P = 128
    CC = C // P  # 4 channel chunks
    MC = M // P  # 8 kv-position chunks
    HP = NUM_HEADS // 2  # 4 head pairs
    NCH = 4  # 128-token n-chunks per n2 half

    from contextlib import ExitStack
    with tile.TileContext(nc) as tc, ExitStack() as ctx:
        consts = ctx.enter_context(tc.tile_pool(name="consts", bufs=1))
        psQK = ctx.enter_context(tc.tile_pool(name="psQK", bufs=2,
                                              space="PSUM"))
        ptp = ctx.enter_context(tc.tile_pool(name="ptp", bufs=42))
        xnp = ctx.enter_context(tc.tile_pool(name="xnp", bufs=2))
        stats = ctx.enter_context(tc.tile_pool(name="stats", bufs=3))
        attnp = ctx.enter_context(tc.tile_pool(name="attnp", bufs=5))
        outs = ctx.enter_context(tc.tile_pool(name="outs", bufs=2))

        # ---- input DMAs, in consumption order ----
        wsh_sb = consts.tile([P, 4, 2, 2, C], fp8)
        wsl_sb = consts.tile([P, 4, 2, 2, C], fp8)
        xph_sb = consts.tile([P, 4, 2, 2, M], fp8)
        xpl_sb = consts.tile([P, 4, 2, 2, M], fp8)

        def xp_load(i):
            sl = np.s_[:, :, :, :, i * 256:(i + 1) * 256]
            nc.sync.dma_start(out=xph_sb[sl], in_=xph_d.ap()[sl])
            nc.sync.dma_start(out=xpl_sb[sl], in_=xpl_d.ap()[sl])

        # upfront loads: everything conv chunks 0-3 + qproj need; the rest is
        # issued mid-stream (below) so the per-chunk xn transposes don't queue
        # on the single HWDGE device behind the whole input working set
        bsr_sb = consts.tile([1, C], bf16)
        nc.sync.dma_start(out=bsr_sb, in_=bsr_d.ap())
        nc.sync.dma_start(out=wsh_sb, in_=wsh_d.ap())
        xp_load(0)
        nc.sync.dma_start(out=wsl_sb, in_=wsl_d.ap())
        wq_sb = consts.tile([P, CC, C], bf16)
        nc.sync.dma_start(
            out=wq_sb, in_=wq_d.ap().rearrange("(cc p) n -> p cc n", p=P))
        xq_sb = consts.tile([P, CC, NQ], bf16)
        nc.sync.dma_start(
            out=xq_sb, in_=xq_d.ap().rearrange("(cc p) n -> p cc n", p=P))
        xp_load(1)
        wk_sb = consts.tile([P, CC, C], bf16)
        nc.sync.dma_start(
            out=wk_sb, in_=wk_d.ap().rearrange("(cc p) n -> p cc n", p=P))
        wv_sb = consts.tile([P, CC, C], bf16)
        wp_sb = consts.tile([P, CC, C], bf16)
        bp_sb = consts.tile([1, C], bf16)
        mask_sb = consts.tile([P, 8], f32)
        maskr_sb = consts.tile([1, 8, P], bf16)

        def late_loads(i):
            # issued between conv chunks, after that chunk's xn transpose
            if i == 0:
                nc.sync.dma_start(
                    out=wv_sb,
                    in_=wv_d.ap().rearrange("(cc p) n -> p cc n", p=P))
            elif i == 1:
                xp_load(2)
            elif i == 2:
                nc.sync.dma_start(
                    out=wp_sb,
                    in_=wp_d.ap().rearrange("(cc p) n -> p cc n", p=P))
            elif i == 3:
                xp_load(3)
            elif i == 4:
                nc.sync.dma_start(out=bp_sb, in_=bp_d.ap())
                nc.sync.dma_start(out=mask_sb, in_=mask_d.ap())
                nc.sync.dma_start(out=maskr_sb, in_=maskr_d.ap())

        ones128 = consts.tile([1, P], bf16)
        nc.vector.memset(ones128, 1.0)
        ones512 = consts.tile([1, C], bf16)
        nc.vector.memset(ones512, 1.0)

        # q/k in fp8, packed per head pair: partitions 0:64 = even head's hd,
        # 64:128 = odd head's. The DoubleRow pair axis (dim 2) carries real
        # data in plane 0 and zeros in plane 1 (the device compiler only
        # accepts DoubleRow at 64/128 partitions, so the 64-deep hd
        # contraction is zero-padded to 64x2).
        qT8_sb = consts.tile([P, HP, 2, NQ], fp8)
        kT8_sb = consts.tile([P, HP, 2, M], fp8)
        nc.gpsimd.memset(qT8_sb[:, :, 1, :], 0.0)
        nc.gpsimd.memset(kT8_sb[:, :, 1, :], 0.0)
        rstd_sb = consts.tile([P, MC], f32)
        v_sb = consts.tile([P, MC, NUM_HEADS, HD + 1], bf16)
        nc.vector.memset(v_sb[:, :, :, HD:HD + 1], 1.0)
        xnT_sb = consts.tile([P, CC, M], bf16)
        # half-size, reused across the two n2 halves (outproj(t) reads the
        # slice before the n2=1 transposes rewrite it)
        attnT_sb = consts.tile([P, CC, NQ // 2], bf16)

        pt_tiles = {}  # (n2, hp, mc) -> SBUF tile of exp(qk)
        attn_tiles = {}  # t -> [128, 512] token-major attn tile (all heads)

        # ---- building blocks ----
        def qk_one(n2, hp, mc):
            """fp8 DoubleRow qk for head pair hp at kv chunk mc -> exp tile."""
            qk = psQK.tile([P, 1024], f32, tag="psQK")
            for par in range(2):
                nc.tensor.matmul(
                    qk[:, par * 512:(par + 1) * 512],
                    kT8_sb[par * 64:(par + 1) * 64, hp, :,
                           mc * P:(mc + 1) * P],
                    qT8_sb[par * 64:(par + 1) * 64, hp, :,
                           n2 * 512:(n2 + 1) * 512],
                    start=True, stop=True, perf_mode=DR,
                    tile_position=(par * 64, 0))
            # xn carries (conv - mu) only; rstd[m] is applied as the exp's
            # per-partition scale (free in the same Activation op)
            pt = ptp.tile([P, 1024], bf16, tag="pt")
            nc.scalar.activation(
                out=pt, in_=qk, func=mybir.ActivationFunctionType.Exp,
                scale=rstd_sb[:, mc:mc + 1])
            pt_tiles[(n2, hp, mc)] = pt

        def qproj(n2, g):
            """bf16 q projection for head pair g; PSUM staged to fp8 qT8
            (scaled up by sqrt(8) to clear the e4m3 subnormal floor)."""
            ps = psQK.tile([P, 1024], f32, tag="psQK")
            for cc in range(CC):
                nc.tensor.matmul(
                    ps[:, 0:512], wq_sb[:, cc, g * P:(g + 1) * P],
                    xq_sb[:, cc, n2 * 512:(n2 + 1) * 512],
                    start=(cc == 0), stop=(cc == CC - 1))
            nc.vector.tensor_scalar_mul(
                out=qT8_sb[:, g, 0, n2 * 512:(n2 + 1) * 512],
                in0=ps[:, 0:512], scalar1=float(QK_BAL))

        def conv_part(i, ij, ps):
            # 3-product fp8 split: hi@hi + hi@lo + lo@hi (lo@lo dropped),
            # each a K=256 DoubleRow pair over the (cch, t) row groups
            for cch in range(2):
                xh = xph_sb[:, ij, cch, :, i * P:(i + 1) * P]
                xl = xpl_sb[:, ij, cch, :, i * P:(i + 1) * P]
                wh = wsh_sb[:, ij, cch, :, :]
                wl = wsl_sb[:, ij, cch, :, :]
                first = (ij == 0 and cch == 0)
                nc.tensor.matmul(ps, xh, wh, start=first, stop=False,
                                 perf_mode=DR)
                nc.tensor.matmul(ps, xh, wl, start=False, stop=False,
                                 perf_mode=DR)
                nc.tensor.matmul(ps, xl, wh, start=False, stop=False,
                                 perf_mode=DR)

        def conv_ln(i, ps):
            nc.tensor.matmul(ps, ones128, bsr_sb, start=False, stop=True)
            st6 = stats.tile([P, 6], f32, tag="st6")
            nc.vector.bn_stats(out=st6, in_=ps)
            mv = stats.tile([P, 2], f32, tag="mv")
            nc.vector.bn_aggr(out=mv, in_=st6)
            # xn = (conv - mu) only: releases the PSUM bank fast. rstd is
            # applied downstream (exp scale AP for the k path, v-copy scale
            # for the v path), so the slow part of the LN sits off the
            # conv-pipeline critical path.
            xn = xnp.tile([P, 512], bf16, tag="xn")
            nc.vector.tensor_scalar_sub(out=xn, in0=ps, scalar1=mv[:, 0:1])
            nc.sync.dma_start(
                out=xnT_sb[:, :, i * P:(i + 1) * P], in_=xn, transpose=True)
            # rstd = rsqrt(var+eps) via Newton on the otherwise-idle Pool
            # engine (linear minimax seed on [1,4] + 3 iterations, all
            # mult/add): the Activation engine then only ever runs Exp, so
            # it never reloads an activation table mid-kernel.
            # var is in x16-scaled conv units -> w = var/256 + eps; the final
            # iteration's affine constants carry the /16 so rstd_sb holds
            # rstd/16, which both consumers (exp scale, v-copy scale) need
            w = stats.tile([P, 1], f32, tag="w")
            nc.gpsimd.tensor_scalar(
                out=w, in0=mv[:, 1:2], scalar1=1.0 / 256.0, scalar2=LN_EPS,
                op0=mybir.AluOpType.mult, op1=mybir.AluOpType.add)
            s = stats.tile([P, 1], f32, tag="s")
            nc.gpsimd.tensor_scalar(
                out=s, in0=w, scalar1=-1.0 / 6.0, scalar2=1.104,
                op0=mybir.AluOpType.mult, op1=mybir.AluOpType.add)
            for it in range(3):
                t = stats.tile([P, 1], f32, tag="nt")
                nc.gpsimd.tensor_tensor(
                    out=t, in0=s, in1=s, op=mybir.AluOpType.mult)
                nc.gpsimd.tensor_tensor(
                    out=t, in0=t, in1=w, op=mybir.AluOpType.mult)
                lastc = 1.0 if it < 2 else 1.0 / 16.0
                nc.gpsimd.tensor_scalar(
                    out=t, in0=t, scalar1=-0.5 * lastc, scalar2=1.5 * lastc,
                    op0=mybir.AluOpType.mult, op1=mybir.AluOpType.add)
                dst = s if it < 2 else rstd_sb[:, i:i + 1]
                nc.gpsimd.tensor_tensor(
                    out=dst, in0=s, in1=t, op=mybir.AluOpType.mult)

        def kproj_g(mc, g, pool):
            """k projection for head pair g at kv chunk mc; PSUM staged
            straight to fp8 kT8."""
            ps = pool.tile([P, P], f32, tag="psK")
            for cc in range(CC):
                nc.tensor.matmul(
                    ps, wk_sb[:, cc, g * P:(g + 1) * P],
                    xnT_sb[:, cc, mc * P:(mc + 1) * P],
                    start=(cc == 0), stop=(cc == CC - 1))
            nc.vector.tensor_scalar_mul(
                out=kT8_sb[:, g, 0, mc * P:(mc + 1) * P], in0=ps,
                scalar1=float(1.0 / QK_BAL))

        def v_proj(mc, pool):
            ps = pool.tile([P, 512], f32, tag="psA")
            for cc in range(CC):
                nc.tensor.matmul(
                    ps, xnT_sb[:, cc, mc * P:(mc + 1) * P], wv_sb[:, cc, :],
                    start=(cc == 0), stop=(cc == CC - 1))
            # rstd[m] folded into the PSUM drain (xn is mean-centered only)
            nc.vector.tensor_scalar_mul(
                out=v_sb[:, mc, :, 0:HD], in0=ps,
                scalar1=rstd_sb[:, mc:mc + 1])

        av_tag = [0]

        def av_accum(n2, hp, nchunk, pool):
            """attn@v for both heads of pair hp at n-chunk nchunk: out
            [128n, 65] PSUM accumulated over all mc, then one per-partition
            divide (x mask) into the token-major attn tile."""
            t = n2 * NCH + nchunk
            at = attn_tiles.get(t)
            if at is None:
                at = attnp.tile([P, C], bf16, tag="attn")
                attn_tiles[t] = at
            for par in range(2):
                h = 2 * hp + par
                av = pool.tile([P, HD + 1], f32, tag=f"av{av_tag[0] % 3}")
                av_tag[0] += 1
                for mc in range(MC):
                    nc.tensor.matmul(
                        av, pt_tiles[(n2, hp, mc)][
                            :, par * 512 + nchunk * P:
                            par * 512 + (nchunk + 1) * P],
                        v_sb[:, mc, h, :],
                        start=(mc == 0), stop=(mc == MC - 1))
                rec = stats.tile([P, 1], f32, tag="rec")
                nc.vector.reciprocal(out=rec, in_=av[:, HD:HD + 1])
                nc.vector.tensor_scalar(
                    out=at[:, h * HD:(h + 1) * HD], in0=av[:, 0:HD],
                    scalar1=rec, scalar2=mask_sb[:, t:t + 1],
                    op0=mybir.AluOpType.mult, op1=mybir.AluOpType.mult)

        def attn_transpose(t):
            nc.sync.dma_start(
                out=attnT_sb[:, :, (t % NCH) * P:(t % NCH + 1) * P],
                in_=attn_tiles.pop(t), transpose=True)

        def out_proj(t, pool):
            ps = pool.tile([P, 512], f32, tag="psO")
            for cc in range(CC):
                nc.tensor.matmul(
                    ps, attnT_sb[:, cc, (t % NCH) * P:(t % NCH + 1) * P],
                    wp_sb[:, cc, :], start=(cc == 0), stop=False)
            # += mask[n] (x) bp[c] outer product (mask already folded into
            # attn, so the bias needs the same mask scale)
            nc.tensor.matmul(ps, maskr_sb[0:1, t, :], bp_sb,
                             start=False, stop=True)
            ot = outs.tile([P, C], bf16, tag="ot")
            nc.vector.tensor_copy(out=ot, in_=ps)
            nc.sync.dma_start(out=out_d[t * P:(t + 1) * P, :], in_=ot)

        # ---- conv phase: conv chunks with the kproj/vproj/qk(n2=0) stream
        # lag-1 behind, exp on Act from chunk 1 onward ----
        with tc.tile_pool(name="psCV", bufs=2, space="PSUM") as psCV:
            # PE warm-up during the initial DMA wait: depends only on memset
            # tiles so it starts at t=0 and carries the p-state ramp before
            # the first conv matmul (sized to end as its inputs land)
            warm_ps = psCV.tile([1, 512], f32, tag="psA")
            NWARM = 7
            for w in range(NWARM):
                nc.tensor.matmul(warm_ps, ones128[0:1, 0:1], ones512,
                                 start=(w == 0), stop=(w == NWARM - 1))

            for i in range(8):
                mk = i - 1  # kv chunk whose xnT landed during chunk i-1
                ps = psCV.tile([P, 512], f32, tag="psA")
                conv_part(i, 0, ps)
                if mk >= 0:
                    kproj_g(mk, 0, psCV)
                    kproj_g(mk, 1, psCV)
                conv_part(i, 1, ps)
                if mk >= 0:
                    kproj_g(mk, 2, psCV)
                    kproj_g(mk, 3, psCV)
                conv_part(i, 2, ps)
                if mk >= 0:
                    qk_one(0, 0, mk)
                    qk_one(0, 1, mk)
                conv_part(i, 3, ps)
                if mk >= 0:
                    v_proj(mk, psCV)
                    qk_one(0, 2, mk)
                    qk_one(0, 3, mk)
                conv_ln(i, ps)
                late_loads(i)
                if i < 2:
                    for g in range(HP):
                        qproj(i, g)
            # drain the pipeline: kv chunk 7
            kproj_g(7, 0, psCV)
            kproj_g(7, 1, psCV)
            kproj_g(7, 2, psCV)
            kproj_g(7, 3, psCV)
            v_proj(7, psCV)
            for hp in range(HP):
                qk_one(0, hp, 7)

        # ---- attention phase ----
        with tc.tile_pool(name="psAT", bufs=1, space="PSUM") as psAT:
            # n2=1 pair-0 + half of pair-1 queue on Act while the PE runs
            # n2=0's attn@v (the remaining qk allocations must wait for the
            # n2=0 pt tiles to be freed by their attn@v reads)
            for mc in range(MC):
                qk_one(1, 0, mc)
            for mc in range(2):
                qk_one(1, 1, mc)
            # n2=0 attn@v nchunk-major, out_proj as each n-chunk completes
            for nchunk in range(NCH):
                for hp in range(HP):
                    av_accum(0, hp, nchunk, psAT)
                attn_transpose(nchunk)
                out_proj(nchunk, psAT)
            # n2=1 pairs 2,3 qk; pair-major attn@v for pairs 0-2
            for mc in range(2, MC):
                qk_one(1, 1, mc)
            for mc in range(MC):
                qk_one(1, 2, mc)
            for nchunk in range(NCH):
                av_accum(1, 0, nchunk, psAT)
            for mc in range(MC):
                qk_one(1, 3, mc)
            for nchunk in range(NCH):
                av_accum(1, 1, nchunk, psAT)
            for nchunk in range(NCH):
                av_accum(1, 2, nchunk, psAT)
            # last pair nchunk-major with out_proj woven in for a short tail
            for nchunk in range(NCH):
                av_accum(1, 3, nchunk, psAT)
                attn_transpose(NCH + nchunk)
                out_proj(NCH + nchunk, psAT)

    nc.compile()
    return nc


def _prep_inputs(x0, x1, mask0, mask1, Wq, Wkv, Wsr, bsr, gamma, beta, Wp, bp):
    """Host-side layout prep -> per-core in_maps."""
    scale = HD ** (-0.5)
    wq = (Wq * scale).astype(BF16)

    def hilo(a):
        hi = a.astype(FP8)
        lo = (a - hi.astype(np.float32)).astype(FP8)
        return hi, lo

    def rowsplit(a, last):
        # [2048-contraction, last] -> [p, ij, cch, t, last]
        return np.ascontiguousarray(
            a.reshape(4, 2, 2, 128, last).transpose(3, 0, 1, 2, 4))

    # conv weights: Wsr[co, ci, i, j] -> per (i,j) lhs [ci, co], x16 so the
    # fp8 hi/lo split clears the e4m3 subnormal floor (LN absorbs the scale)
    wsr = np.stack([Wsr[:, :, ij // 2, ij % 2].T for ij in range(4)]) * 16.0
    wsr_hi, wsr_lo = hilo(wsr.reshape(2048, C))
    wsr_hi = rowsplit(wsr_hi, C)
    wsr_lo = rowsplit(wsr_lo, C)
    bsr_r = (bsr * 16.0).reshape(1, C).astype(BF16)
    # fold LN gamma/beta into Wkv; drop the k bias (softmax-invariant) and
    # fold the v bias through the output projection: y = attn@Wp + bv@Wp + bp
    Wkv_f = gamma[:, None] * Wkv
    bkv = beta @ Wkv
    wk = np.ascontiguousarray(Wkv_f[:, :C]).astype(BF16)
    wv = Wkv_f[:, C:].astype(BF16)
    bv = bkv[C:]
    wp = Wp.astype(BF16)
    bp_r = (bp + bv @ Wp).reshape(1, C).astype(BF16)

    shared = dict(wq=wq, wsr_hi=wsr_hi, wsr_lo=wsr_lo, bsr_r=bsr_r, wk=wk,
                  wv=wv, wp=wp, bp_r=bp_r)

    xT = [np.ascontiguousarray(x[0].T).astype(BF16) for x in (x0, x1)]
    # patch-major gather for the conv (from full-precision x):
    # xp[ij][c, oh*32+ow] = xT[c, 128*oh+64*i+2*ow+j]
    xp8 = []
    for b in range(2):
        vv = np.ascontiguousarray((x0 if b == 0 else x1)[0].T).reshape(
            C, 32, 2, 32, 2)
        xpb = np.stack([
            vv[:, :, ij // 2, :, ij % 2].reshape(C, M) for ij in range(4)])
        hi, lo = hilo(xpb.reshape(4 * C, M))
        xp8.append((rowsplit(hi, M), rowsplit(lo, M)))
    masks = (mask0, mask1)
    in_maps = []
    for core in range(N_CORES):
        b, s = core // 4, core % 4
        m = dict(shared)
        m["xp_hi"], m["xp_lo"] = xp8[b]
        m["xqT"] = np.ascontiguousarray(xT[b][:, s * NQ:(s + 1) * NQ])
        msk = np.asarray(masks[b][0, s * NQ:(s + 1) * NQ], np.float32)
        m["mask_s"] = np.ascontiguousarray(
            msk.reshape(NQ // 128, 128).T).astype(np.float32)
        m["mask_r"] = np.ascontiguousarray(
            msk.reshape(1, NQ // 128, 128)).astype(BF16)
        in_maps.append(m)
    return in_maps


def kernel(x0, x1, mask0, mask1, Wq, Wkv, Wsr, bsr, gamma, beta, Wp, bp,
           H=64, W=64, _trace=False):
    global _compiled
    x0 = np.asarray(x0, np.float32)
    x1 = np.asarray(x1, np.float32)
    mask0 = np.asarray(mask0, np.float32)
    mask1 = np.asarray(mask1, np.float32)
    assert x0.shape == (B, N, C) and int(H) == 64 and int(W) == 64

    from concourse.bass_utils import run_bass_kernel_spmd

    if _compiled is None:
        _compiled = _build()
    nc = _compiled

    in_maps = _prep_inputs(
        x0, x1, mask0, mask1,
        np.asarray(Wq, np.float32), np.asarray(Wkv, np.float32),
        np.asarray(Wsr, np.float32), np.asarray(bsr, np.float32),
        np.asarray(gamma, np.float32), np.asarray(beta, np.float32),
        np.asarray(Wp, np.float32), np.asarray(bp, np.float32))

    kw = {}
    if _trace:
        kw = dict(trace=True, trace_cores=[0])
    try:
        res = run_bass_kernel_spmd(nc, in_maps, list(range(N_CORES)), **kw)
    except ModuleNotFoundError:
        res = run_bass_kernel_spmd(nc, in_maps, list(range(N_CORES)))

    o0 = np.concatenate(
        [res.results[i]["out"].astype(np.float32) for i in range(4)], axis=0)
    o1 = np.concatenate(
        [res.results[i]["out"].astype(np.float32) for i in range(4, 8)],
        axis=0)
    keep0 = (mask0[0] >= MASK_THRESHOLD)[:, None]
    keep1 = (mask1[0] >= MASK_THRESHOLD)[:, None]
    y0 = np.where(keep0, o0, o1)[None]
    y1 = np.where(keep1, o1, o0)[None]
    out = np.stack([y0, y1]).astype(np.float32)
    if _trace:
        kernel._last_result = res
    return out


kernel._last_result = None
